# revision 11
# baseline (speedup 1.0000x reference)
"""Trainium2 Bass kernel for a SwiGLU-style feed-forward block.

reference:
    gate = x @ w1.T ; up = x @ w2.T ; h = silu(gate) * up ; out = h @ w3.T
    x: [4, 2048, 2048] f32, w1/w2: [8192, 2048] f32, w3: [2048, 8192] f32

Strategy: pure data-parallel over the 8192 tokens -- each of the 8
NeuronCores gets ONE chunk of 1024 tokens and the full weights (loaded
once, not per-chunk).  All tensors are pre-packed + cast to bf16 on the
host into the exact tiled order the kernel consumes, so every DMA is
contiguous (4-16KB runs per partition line) and the TensorEngine
contraction dim always sits on SBUF partitions.

Per core (tokens split into two 512-token groups g0/g1 for PSUM):
    warmup: ~10 scratch matmuls at t=0 keep the PE busy during the
            initial DMA fill and hold the HAM clock-gate at 2.4 GHz.
    phase A: per h-tile (128 rows of H): pg0,pg1 = w1-slab^T @ x(g0/g1)
             pu0,pu1 = w2-slab^T @ x; silu on ACT, mul on DVE -> hT bf16
    phase B: per e-tile (128 rows of E): po0/po1 accumulate over all
             64 h-subtiles, interleaved MM-by-MM so the w3 quarter
             slabs are released early for the next e-tile's prefetch.
Output is outT [16, 128, 1024] f32 per core; the host reassembles.
"""

import json

import numpy as np
import ml_dtypes

import concourse.bass as bass
import concourse.mybir as mybir
import concourse.tile as tile
from concourse.vector_clock import ScopedClock
from concourse.bass_utils import run_bass_kernel_spmd

# ---------------------------------------------------------------- shapes
N_CORES = 8
EMB = 2048          # E
HID = 8192          # H
T_TOTAL = 8192      # B*S tokens
T_SHARD = T_TOTAL // N_CORES   # 1024 tokens per core
TG = 512                       # tokens per PSUM group (2 groups)
E_SUB = EMB // 128             # 16 contraction subtiles for phase A
H_SUB = HID // 128             # 64 contraction subtiles for phase B
H_TILES = HID // 128           # 64 h-tiles in phase A
ET = EMB // 128                # 16 e-tiles in phase B
N_WARM = 8                     # scratch matmuls before real work

CDT = mybir.dt.bfloat16        # compute dtype on the PE
NP_CDT = ml_dtypes.bfloat16

P = 128
F32 = mybir.dt.float32


class _TileContextSplitWait(tile.TileContext):
    """The walrus build in this environment rejects >1 sync-wait on a
    CTRL (Drain) instruction.  Split the kernel-tail drain's waits into
    single-wait nops emitted just before it."""

    def _drain_and_barrier(self, tick_clock, wait_clock):
        probe = self.nc.sync.nop(nofuse=True)
        wait_clock.add_sem_waits(
            probe.ins, ScopedClock({None: tick_clock.global_clock})
        )
        si = probe.ins.sync_info
        if si is not None and len(si.on_wait) > 1:
            waits = list(si.on_wait)
            probe.ins.sync_info = mybir.SyncInfo(
                on_wait=waits[:1], on_update=list(si.on_update)
            )
            for w in waits[1:]:
                n = self.nc.sync.nop(nofuse=True)
                n.ins.sync_info = mybir.SyncInfo(on_wait=[w], on_update=[])
        self.nc.sync.drain()
        self.nc.all_engine_barrier()
        assert self.sems is not None
        popped = self.nc._tile_sem_poison_stack.pop()
        assert popped is self._sem_poison
        self.nc.clear_and_free_semaphores(list(self.sems.allocated().values()))
        self.nc.all_engine_barrier()


def _split_multi_waits(bir_bytes):
    """The walrus build here accepts at most one sync-wait command per
    instruction (setupSyncWait raises 'Too many sync wait commands').
    Tile attaches however many the dependence analysis needs, so move
    extra waits onto NoOp instructions inserted just before, on the same
    engine's stream -- semantically identical, codegen-compatible."""
    bir = json.loads(bir_bytes)
    for fn in bir["functions"]:
        for blk in fn["blocks"]:
            insts = blk.get("instructions")
            if not insts:
                continue
            out = []
            changed = False
            for inst in insts:
                si = inst.get("sync_info")
                waits = (si or {}).get("on_wait") or []
                if len(waits) > 1:
                    changed = True
                    for j, w in enumerate(waits[:-1]):
                        out.append(
                            {
                                "debug": inst.get("debug"),
                                "engine": inst["engine"],
                                "ins": [],
                                "name": f"{inst['name']}-w{j}",
                                "opcode": "NoOp",
                                "outs": [],
                                "sync_info": {"on_update": [], "on_wait": [w]},
                            }
                        )
                    si["on_wait"] = waits[-1:]
                out.append(inst)
            if changed:
                blk["instructions"] = out
    return json.dumps(bir).encode()


def _build_nc():
    nc = bass.Bass(target_bir_lowering=False)

    # host-packed inputs: every DMA the kernel issues is a contiguous
    # (or 1-descriptor-per-partition) block of these
    xd = nc.dram_tensor("xp", [2, P, E_SUB, TG], CDT, kind="ExternalInput")
    w1d = nc.dram_tensor("w1p", [H_TILES, P, E_SUB, P], CDT, kind="ExternalInput")
    w2d = nc.dram_tensor("w2p", [H_TILES, P, E_SUB, P], CDT, kind="ExternalInput")
    w3d = nc.dram_tensor("w3p", [ET, P, H_SUB, P], CDT, kind="ExternalInput")
    outd = nc.dram_tensor("outp", [ET, P, T_SHARD], F32, kind="ExternalOutput")

    xv = xd[:]
    w1v = w1d[:]
    w2v = w2d[:]
    w3v = w3d[:]
    outv = outd[:]

    with _TileContextSplitWait(nc) as tc:
        with (
            tc.tile_pool(name="xpool", bufs=1) as xpool,
            tc.tile_pool(name="hpool", bufs=1) as hpool,
            tc.tile_pool(name="wp", bufs=2) as wp,
            tc.tile_pool(name="w3pool", bufs=4) as w3pool,
            tc.tile_pool(name="scr", bufs=1) as scr,
            tc.tile_pool(name="slp", bufs=2) as slp,
            tc.tile_pool(name="op", bufs=2) as op,
            tc.tile_pool(name="ps", bufs=2, space="PSUM") as ps,
        ):
            # ---------------- warmup: PE busy from t=0, no DMA deps
            wsc = scr.tile([P, TG], CDT, name="wsc")
            nc.vector.memset(wsc[:], 0.0)
            wps = ps.tile([P, TG], F32, name="wps", tag="pg0")
            for i in range(N_WARM):
                nc.tensor.matmul(
                    wps[:],
                    wsc[:, 0:P],
                    wsc[:],
                    start=(i == 0),
                    stop=(i == N_WARM - 1),
                )

            # ---------------- resident tiles
            xs = xpool.tile([P, 2, E_SUB, TG], CDT, name="xs")
            ht = hpool.tile([P, H_SUB, T_SHARD], CDT, name="ht")

            # ---------------- startup DMAs, hand-ordered on the ring.
            # Token group g0's x lands first so the first h-tile's
            # pg0/pu0 groups start after ~2.5MB of fill, not 4.5MB.
            w1s0 = wp.tile([P, E_SUB, P], CDT, name="w1s")
            nc.sync.dma_start(w1s0[:], w1v[0])
            for q in range(8):
                nc.sync.dma_start(
                    xs[:, 0, 2 * q : 2 * q + 2, :], xv[0][:, 2 * q : 2 * q + 2, :]
                )
            w2s0 = wp.tile([P, E_SUB, P], CDT, name="w2s")
            nc.sync.dma_start(w2s0[:], w2v[0])
            for q in range(8):
                nc.sync.dma_start(
                    xs[:, 1, 2 * q : 2 * q + 2, :], xv[1][:, 2 * q : 2 * q + 2, :]
                )

            w3_et0 = []

            def load_w3_quarters(et, eng):
                tiles = []
                for q in range(4):
                    t = w3pool.tile([P, E_SUB, P], CDT, name="w3q")
                    eng.dma_start(
                        t[:], w3v[et][:, E_SUB * q : E_SUB * (q + 1), :]
                    )
                    tiles.append(t)
                return tiles

            # ---------------- phase A: hT = silu(x@w1T) * (x@w2T)
            for h in range(H_TILES):
                if h == 0:
                    w1s, w2s = w1s0, w2s0
                else:
                    w1s = wp.tile([P, E_SUB, P], CDT, name="w1s")
                    nc.sync.dma_start(w1s[:], w1v[h])
                    w2s = wp.tile([P, E_SUB, P], CDT, name="w2s")
                    nc.sync.dma_start(w2s[:], w2v[h])
                if h == 1:
                    # et0's w3 rides the idle ACT ring here, long
                    # before phase B needs it
                    w3_et0 = load_w3_quarters(0, nc.scalar)

                # group order g0-first: pg0, pu0, pg1, pu1 — the first
                # h-tile can run on half the x fill; silu/mul of each
                # group overlaps the next group's matmuls
                pg0 = ps.tile([P, TG], F32, name="pg0", tag="pg0")
                pu0 = ps.tile([P, TG], F32, name="pu0", tag="pu0")
                pg1 = ps.tile([P, TG], F32, name="pg1", tag="pg1")
                pu1 = ps.tile([P, TG], F32, name="pu1", tag="pu1")
                for e in range(E_SUB):
                    nc.tensor.matmul(
                        pg0[:], w1s[:, e, :], xs[:, 0, e, :],
                        start=(e == 0), stop=(e == E_SUB - 1),
                    )
                for e in range(E_SUB):
                    nc.tensor.matmul(
                        pu0[:], w2s[:, e, :], xs[:, 0, e, :],
                        start=(e == 0), stop=(e == E_SUB - 1),
                    )
                sl0 = slp.tile([P, TG], CDT, name="sl0")
                nc.scalar.activation(
                    sl0[:], pg0[:], mybir.ActivationFunctionType.Silu
                )
                for e in range(E_SUB):
                    nc.tensor.matmul(
                        pg1[:], w1s[:, e, :], xs[:, 1, e, :],
                        start=(e == 0), stop=(e == E_SUB - 1),
                    )
                nc.vector.tensor_mul(ht[:, h, 0:TG], sl0[:], pu0[:])
                for e in range(E_SUB):
                    nc.tensor.matmul(
                        pu1[:], w2s[:, e, :], xs[:, 1, e, :],
                        start=(e == 0), stop=(e == E_SUB - 1),
                    )
                sl1 = slp.tile([P, TG], CDT, name="sl1")
                nc.scalar.activation(
                    sl1[:], pg1[:], mybir.ActivationFunctionType.Silu
                )
                nc.vector.tensor_mul(ht[:, h, TG:T_SHARD], sl1[:], pu1[:])

            # ---------------- phase B: outT = sum_h w3T^T @ hT
            # output DMAs go on the ACT ring (idle in phase B) so they
            # never delay the sync ring's w3 prefetch stream
            for et in range(ET):
                w3q = w3_et0 if et == 0 else load_w3_quarters(et, nc.sync)
                po0 = ps.tile([P, TG], F32, name="po0", tag="pg0")
                po1 = ps.tile([P, TG], F32, name="po1", tag="pg1")
                if et < ET - 1:
                    # interleave the two token groups so each w3
                    # quarter's last read is at h=16q+15, freeing its
                    # slot early for the next e-tile's prefetch
                    for h in range(H_SUB):
                        w3t = w3q[h // E_SUB]
                        nc.tensor.matmul(
                            po0[:], w3t[:, h % E_SUB, :], ht[:, h, 0:TG],
                            start=(h == 0), stop=(h == H_SUB - 1),
                        )
                        nc.tensor.matmul(
                            po1[:], w3t[:, h % E_SUB, :], ht[:, h, TG:T_SHARD],
                            start=(h == 0), stop=(h == H_SUB - 1),
                        )
                    ot0 = op.tile([P, TG], F32, name="ot")
                    nc.vector.tensor_copy(ot0[:], po0[:])
                    nc.scalar.dma_start(outv[et][:, 0:TG], ot0[:])
                    ot1 = op.tile([P, TG], F32, name="ot")
                    nc.vector.tensor_copy(ot1[:], po1[:])
                    nc.scalar.dma_start(outv[et][:, TG:T_SHARD], ot1[:])
                else:
                    # last e-tile: run the groups sequentially so the
                    # first copy+DMA overlaps the second group's MMs,
                    # shortening the kernel tail
                    for h in range(H_SUB):
                        w3t = w3q[h // E_SUB]
                        nc.tensor.matmul(
                            po0[:], w3t[:, h % E_SUB, :], ht[:, h, 0:TG],
                            start=(h == 0), stop=(h == H_SUB - 1),
                        )
                    ot0 = op.tile([P, TG], F32, name="ot")
                    nc.vector.tensor_copy(ot0[:], po0[:])
                    nc.scalar.dma_start(outv[et][:, 0:TG], ot0[:])
                    for h in range(H_SUB):
                        w3t = w3q[h // E_SUB]
                        nc.tensor.matmul(
                            po1[:], w3t[:, h % E_SUB, :], ht[:, h, TG:T_SHARD],
                            start=(h == 0), stop=(h == H_SUB - 1),
                        )
                    ot1 = op.tile([P, TG], F32, name="ot")
                    nc.vector.tensor_copy(ot1[:], po1[:])
                    nc.scalar.dma_start(outv[et][:, TG:T_SHARD], ot1[:])

    fixed = _split_multi_waits(bass.Bass.to_json_bytes(nc))
    nc.to_json_bytes = lambda: fixed
    return nc


_nc_cache = None


def _get_nc():
    global _nc_cache
    if _nc_cache is None:
        _nc_cache = _build_nc()
    return _nc_cache


def _prep_inputs(x, w1, w2, w3):
    # x: [B,S,E] f32 -> per core [128p, 16s, 1024t] bf16,
    # xp[p, s, t] = x[t0+t, s*128+p]
    xt = x.reshape(T_TOTAL, EMB).astype(NP_CDT)
    # w1/w2: [H, E] -> [64ht, 128p, 16s, 128j], w[ht,p,s,j] = w1[ht*128+j, s*128+p]
    w1p = np.ascontiguousarray(
        w1.astype(NP_CDT).reshape(H_TILES, P, E_SUB, P).transpose(0, 3, 2, 1)
    )
    w2p = np.ascontiguousarray(
        w2.astype(NP_CDT).reshape(H_TILES, P, E_SUB, P).transpose(0, 3, 2, 1)
    )
    # w3: [E, H] -> [16et, 128p, 64hs, 128j], w3p[et,p,hs,j] = w3[et*128+j, hs*128+p]
    w3p = np.ascontiguousarray(
        w3.astype(NP_CDT).reshape(ET, P, H_SUB, P).transpose(0, 3, 2, 1)
    )
    in_maps = []
    for i in range(N_CORES):
        xc = xt[i * T_SHARD : (i + 1) * T_SHARD]  # [1024, 2048]
        xp = np.ascontiguousarray(
            xc.reshape(2, TG, E_SUB, P).transpose(0, 3, 2, 1)
        )  # [2, 128, 16, 512]
        in_maps.append({"xp": xp, "w1p": w1p, "w2p": w2p, "w3p": w3p})
    return in_maps


def kernel(x, w1, w2, w3, scale_x=None, _trace=False):
    x = np.asarray(x, np.float32)
    w1 = np.asarray(w1, np.float32)
    w2 = np.asarray(w2, np.float32)
    w3 = np.asarray(w3, np.float32)

    nc = _get_nc()
    in_maps = _prep_inputs(x, w1, w2, w3)
    res = run_bass_kernel_spmd(nc, in_maps, list(range(N_CORES)), trace=_trace)

    outt = np.concatenate(
        [
            np.asarray(res.results[i]["outp"]).reshape(EMB, T_SHARD)
            for i in range(N_CORES)
        ],
        axis=1,
    )  # [E, T_total]
    out = np.ascontiguousarray(outt.T).reshape(4, 2048, EMB).astype(np.float32)
    if _trace:
        kernel.last_results = res
    return out


if __name__ == "__main__":
    rng = np.random.default_rng(0)
    x = rng.standard_normal((4, 2048, EMB), dtype=np.float32)
    w1 = (rng.standard_normal((HID, EMB), dtype=np.float32) * 0.03).astype(
        np.float32
    )
    w2 = (rng.standard_normal((HID, EMB), dtype=np.float32) * 0.03).astype(
        np.float32
    )
    w3 = (rng.standard_normal((EMB, HID), dtype=np.float32) * 0.015).astype(
        np.float32
    )
    out = kernel(x, w1, w2, w3)
    print("out", out.shape, out.dtype, float(np.abs(out).mean()))
